# revision 17
# baseline (speedup 1.0000x reference)
"""Trainium2 Bass kernel for nn_Attention (single-query attention variant).

Reference computation (per batch b):
  q = ff @ Wq + bq                      [1, C]   (C=1024, H=16 heads, d=64)
  k = x @ Wk + bk                       [N, C]
  v = x @ Wv + bv                       [N, C]
  s[h, n]    = (1/sqrt(d)) * sum_d q[hd] k[n, hd]
  attn[h, n] = softmax_n(s[h, :])
  out[n, c]  = attn[h(c), n] * v[n, c]

Key algebraic folds used here:
  * k is only consumed through per-head dot products with q, so
    s = x @ Wt + const_h with Wt[c, h] = scale * sum_d Wk[c, hd] q[hd].
    This removes the entire x@Wk matmul (half the FLOPs).
  * const_h (from bk and q) is constant along n, and softmax is
    shift-invariant, so bk drops out of BOTH outputs exactly.
  * softmax needs no max-subtraction: |s| <= ~15 here, exp is safe in fp32.
  * Cross-partition (token) sums of exp(s) ride the PE as ones-vector
    matmuls accumulating into one PSUM bank across all 32 token tiles.

Distribution: pure data-parallel over batch B=16 across 8 cores (2 each).
Matmuls run in float32r (full PE rate, ~1.6e-4 rel err measured on HW).
"""

import sys

sys.path.insert(0, "/opt/trn_rl_repo")

from contextlib import ExitStack

import numpy as np

import concourse.bass as bass
import concourse.tile as tile
from concourse import bacc, mybir
from concourse.bass import ts
from concourse.bass_utils import run_bass_kernel_spmd
from concourse.masks import make_identity

F32 = mybir.dt.float32
F32R = mybir.dt.float32r
F16 = mybir.dt.float16

B, N, C, H, D = 16, 4096, 1024, 16, 64
NCORES = 8
BL = B // NCORES  # batches per core
PT = 128  # tokens per tile
KC = C // 128  # contraction chunks
SEG = 2  # tiles per out-DMA group
SCALE = float(D) ** -0.5


def build_kernel(n_tok: int = N, reps: int = 1):
    """Build the per-core SPMD program. n_tok can be reduced for simulation.

    reps > 1 re-emits the steady-state phase; used to measure marginal
    per-iteration hardware time (fixed dispatch overhead cancels).
    """
    nt = n_tok // PT  # token tiles per batch
    ns = nt // SEG

    nc = bacc.Bacc("TRN2", target_bir_lowering=False, debug=False)
    x_d = nc.dram_tensor("x_sh", [BL, n_tok, C], F32, kind="ExternalInput")
    ff_d = nc.dram_tensor("ff_sh", [BL, C], F32, kind="ExternalInput")
    wq_d = nc.dram_tensor("wq", [C, C], F32, kind="ExternalInput")
    bq_d = nc.dram_tensor("bq", [C], F32, kind="ExternalInput")
    wk_d = nc.dram_tensor("wk", [C, C], F32, kind="ExternalInput")
    wv_d = nc.dram_tensor("wv", [C, C], F32, kind="ExternalInput")
    bv_d = nc.dram_tensor("bv", [C], F32, kind="ExternalInput")
    out_d = nc.dram_tensor("out_sh", [BL, n_tok, C], F32, kind="ExternalOutput")
    at_d = nc.dram_tensor("attn_sh", [BL, H, 1, n_tok], F32, kind="ExternalOutput")

    with tile.TileContext(nc) as tc, ExitStack() as ctx:
        _body(ctx, tc, x_d, ff_d, wq_d, bq_d, wk_d, wv_d, bv_d, out_d, at_d, nt, ns, reps)
    nc.compile()
    return nc


def _body(ctx, tc, x_d, ff_d, wq_d, bq_d, wk_d, wv_d, bv_d, out_d, at_d, nt, ns, reps):
    n_tok = nt * PT
    nc = tc.nc

    singles = ctx.enter_context(tc.tile_pool(name="singles", bufs=1))
    prol = ctx.enter_context(tc.tile_pool(name="prol", bufs=2))
    xin = ctx.enter_context(tc.tile_pool(name="xin", bufs=2))
    xtp = ctx.enter_context(tc.tile_pool(name="xtp", bufs=2))
    outp = ctx.enter_context(tc.tile_pool(name="outp", bufs=2))
    attp = ctx.enter_context(tc.tile_pool(name="attp", bufs=2))
    stash = ctx.enter_context(tc.tile_pool(name="stash", bufs=nt + 2))

    ps_xt = ctx.enter_context(tc.tile_pool(name="ps_xt", bufs=2, space="PSUM"))
    ps_v = ctx.enter_context(tc.tile_pool(name="ps_v", bufs=3, space="PSUM"))
    ps_s = ctx.enter_context(tc.tile_pool(name="ps_s", bufs=1, space="PSUM"))
    ps_sum = ctx.enter_context(tc.tile_pool(name="ps_sum", bufs=1, space="PSUM"))
    ps_at = ctx.enter_context(tc.tile_pool(name="ps_at", bufs=1, space="PSUM"))

    # ---------------- constants ----------------
    ident = singles.tile([128, 128], F32)
    make_identity(nc, ident)
    ident_r = singles.tile([128, 128], F32R)
    nc.scalar.copy(ident_r, ident)
    ones_col = singles.tile([128, 1], F32)
    nc.vector.memset(ones_col, 1.0)

    bv_ap = bv_d.ap()
    bv_rep = singles.tile([128, C], F32)
    nc.gpsimd.dma_start(
        out=bv_rep,
        in_=bass.AP(tensor=bv_ap.tensor, offset=bv_ap.offset, ap=[[0, 128], *bv_ap.ap]),
    )

    # Wv resident in SBUF, f32r, laid out [c_in_chunk(part), chunk, c_out]
    wv_sb = singles.tile([128, KC, C], F32R)
    nc.sync.dma_start(
        out=wv_sb, in_=wv_d.ap().rearrange("(k p) n -> p k n", p=128).bitcast(F32R)
    )

    # ---------------- prologue: q then Wt (per batch) ----------------
    # ff -> ffT [c(part), chunk, batch]
    ff_sb = singles.tile([BL, C], F32)
    nc.sync.dma_start(out=ff_sb, in_=ff_d.ap())
    ffT_ps = ps_xt.tile([128, KC, BL], F32, tag="xtps")
    for k in range(KC):
        nc.tensor.transpose(ffT_ps[:, k, :], ff_sb[:, ts(k, 128)], ident[:BL, :BL])
    ffT_sb = singles.tile([128, KC, BL], F32R)
    nc.scalar.copy(ffT_sb, ffT_ps)

    # q = ff @ Wq + bq   (accumulated over chunks, f32r full-rate)
    q_ps0 = ps_v.tile([BL, 512], F32, tag="v")
    q_ps1 = ps_v.tile([BL, 512], F32, tag="v")
    q_ps = [q_ps0, q_ps1]
    wq_view = wq_d.ap().rearrange("(k p) n -> k p n", p=128)
    for k in range(KC):
        wq_t = prol.tile([128, C], F32R, tag="wstream")
        nc.sync.dma_start(out=wq_t, in_=wq_view[k].bitcast(F32R))
        for h2 in range(2):
            nc.tensor.matmul(
                q_ps[h2],
                ffT_sb[:, k, :],
                wq_t[:, ts(h2, 512)],
                start=(k == 0),
                stop=(k == KC - 1),
            )
    bq_ap = bq_d.ap()
    bq_sb = singles.tile([BL, C], F32)
    nc.gpsimd.dma_start(
        out=bq_sb,
        in_=bass.AP(tensor=bq_ap.tensor, offset=bq_ap.offset, ap=[[0, BL], *bq_ap.ap]),
    )
    q_sb = singles.tile([BL, C], F32)
    for h2 in range(2):
        nc.vector.tensor_add(q_sb[:, ts(h2, 512)], q_ps[h2], bq_sb[:, ts(h2, 512)])

    # bounce q through DRAM to replicate it across all 128 partitions
    dramp = ctx.enter_context(tc.tile_pool(name="dram", bufs=1, space="DRAM"))
    q_dram = dramp.tile([BL, C], F32)
    nc.sync.dma_start(out=q_dram, in_=q_sb)
    qrepp = ctx.enter_context(tc.tile_pool(name="qrepp", bufs=1))

    # Wt[c, h] = SCALE * sum_d Wk[c, h*D+d] * q[h*D+d], per batch.
    # Computed per batch lazily (batch 0 first so phase A can start early).
    wk_view = wk_d.ap().rearrange("(k p) n -> k p n", p=128)
    wt_sb = singles.tile([128, KC, BL, H], F32R)

    def compute_wt(b):
        q_rep = qrepp.tile([128, C], F32, tag="qrep")
        nc.gpsimd.dma_start(
            out=q_rep,
            in_=bass.AP(
                tensor=q_dram.tensor,
                offset=q_dram.offset + b * C,
                ap=[[0, 128], q_dram.ap[1]],
            ),
        )
        for k in range(KC):
            wk_t = prol.tile([128, C], F32, tag="wstream")
            nc.sync.dma_start(out=wk_t, in_=wk_view[k])
            tmp = prol.tile([128, C], F32, tag="wtmp")
            nc.vector.tensor_mul(tmp, wk_t, q_rep)
            red = prol.tile([128, H], F32, tag="wtred")
            nc.vector.reduce_sum(
                out=red,
                in_=tmp.rearrange("p (h d) -> p h d", d=D),
                axis=mybir.AxisListType.X,
            )
            # scale + round-to-f32r in one ACT pass
            nc.scalar.mul(wt_sb[:, k, b, :], red, SCALE)

    # exp(s) stash (f32) and v stash (f16) live across a batch
    exp_sb = singles.tile([128, BL, nt, H], F32)
    attnp = ctx.enter_context(tc.tile_pool(name="attnp", bufs=1))
    rinv = singles.tile([1, BL, H], F32)
    rrep = singles.tile([128, BL, H], F32)

    x_view = x_d.ap().rearrange("b (s j p) c -> b s p j c", p=128, j=SEG)
    out_view = out_d.ap().rearrange("b (s j p) c -> b s p j c", p=128, j=SEG)

    for rep in range(reps):
      for b in range(BL):
        if rep == 0:
            compute_wt(b)
        stash_tiles = []
        sums = ps_sum.tile([1, H], F32, tag="sums")

        # ---------------- phase A: stream x, make v + exp(s) ----------------
        for s in range(ns):
            x_t = xin.tile([128, SEG, C], F32R)
            nc.sync.dma_start(out=x_t, in_=x_view[b, s].bitcast(F32R))
            for j in range(SEG):
                t = s * SEG + j
                # transpose x tile -> xT [c(part), tok]
                xt_sb = xtp.tile([128, KC, 128], F32R, tag="xt")
                for g in range(2):
                    xt_ps = ps_xt.tile([128, 4, 128], F32R, tag="xtps")
                    for kk in range(4):
                        nc.tensor.transpose(
                            xt_ps[:, kk, :],
                            x_t[:, j, ts(g * 4 + kk, 128)],
                            ident_r,
                        )
                    nc.scalar.copy(xt_sb[:, g * 4 : g * 4 + 4, :], xt_ps)

                # v = x @ Wv (psum, f32r) and s = x @ Wt
                v0 = ps_v.tile([128, 512], F32, tag="v")
                v1 = ps_v.tile([128, 512], F32, tag="v")
                sc = ps_s.tile([128, H], F32, tag="sc")
                for k in range(KC):
                    st = xt_sb[:, k, :]
                    first, last = k == 0, k == KC - 1
                    nc.tensor.matmul(v0, st, wv_sb[:, k, 0:512], start=first, stop=last)
                    nc.tensor.matmul(v1, st, wv_sb[:, k, 512:C], start=first, stop=last)
                    nc.tensor.matmul(
                        sc, st, wt_sb[:, k, b, :], start=first, stop=last
                    )

                # exp(s) -> stash; sum over tokens via ones-matmul into PSUM
                nc.scalar.activation(
                    out=exp_sb[:, b, t, :], in_=sc, func=mybir.ActivationFunctionType.Exp
                )
                nc.tensor.matmul(
                    sums,
                    ones_col,
                    exp_sb[:, b, t, :],
                    start=(t == 0),
                    stop=(t == nt - 1),
                )

                # v + bv -> f16 stash
                st_t = stash.tile([128, C], F16, tag="stash")
                nc.vector.tensor_add(st_t[:, 0:512], v0, bv_rep[:, 0:512])
                nc.vector.tensor_add(st_t[:, 512:C], v1, bv_rep[:, 512:C])
                stash_tiles.append(st_t)

        # ---------------- phase B: normalize, emit attn + out ----------------
        attn_sb = attnp.tile([H, n_tok], F32, tag="attn")
        nc.vector.reciprocal(rinv[:, b, :], sums)
        nc.gpsimd.partition_broadcast(rrep[:, b, :], rinv[:, b, :])

        for s in range(ns):
            o_t = outp.tile([128, SEG, C], F32)
            for j in range(SEG):
                t = s * SEG + j
                at = attp.tile([128, H], F32, tag="at")
                nc.vector.tensor_mul(at, exp_sb[:, b, t, :], rrep[:, b, :])
                atT = ps_at.tile([H, 128], F32, tag="atT")
                nc.tensor.transpose(atT, at, ident)
                nc.scalar.copy(attn_sb[:, ts(t, PT)], atT)
                at_bc = bass.AP(
                    tensor=at.tensor, offset=at.offset, ap=[*at.ap, [0, D]]
                )
                nc.vector.tensor_mul(
                    o_t[:, j, :].rearrange("p (h d) -> p h d", d=D),
                    stash_tiles[t].rearrange("p (h d) -> p h d", d=D),
                    at_bc,
                )
            nc.sync.dma_start(out=out_view[b, s], in_=o_t)
        nc.sync.dma_start(out=at_d.ap()[b, :, 0, :], in_=attn_sb)


_NC_CACHE = {}


def _get_nc(n_tok=N, reps=1):
    key = (n_tok, reps)
    if key not in _NC_CACHE:
        _NC_CACHE[key] = build_kernel(n_tok, reps)
    return _NC_CACHE[key]


def run(inputs, n_tok=N, trace=False):
    """Shard inputs, run on 8 cores, gather. Returns (out, attn, results)."""
    x = np.ascontiguousarray(inputs["x"], dtype=np.float32)
    ff = np.ascontiguousarray(inputs["fore_feature"], dtype=np.float32)
    wq = np.ascontiguousarray(inputs["Wq"], dtype=np.float32)
    bq = np.ascontiguousarray(inputs["bq"], dtype=np.float32)
    wk = np.ascontiguousarray(inputs["Wk"], dtype=np.float32)
    wv = np.ascontiguousarray(inputs["Wv"], dtype=np.float32)
    bv = np.ascontiguousarray(inputs["bv"], dtype=np.float32)

    nc = _get_nc(n_tok)
    in_maps = []
    for i in range(NCORES):
        sl = slice(i * BL, (i + 1) * BL)
        in_maps.append(
            {
                "x_sh": np.ascontiguousarray(x[sl, :n_tok]),
                "ff_sh": np.ascontiguousarray(ff[sl, 0]),
                "wq": wq,
                "bq": bq,
                "wk": wk,
                "wv": wv,
                "bv": bv,
            }
        )
    res = run_bass_kernel_spmd(
        nc, in_maps, core_ids=list(range(NCORES)), trace=trace
    )
    out = np.concatenate([r["out_sh"] for r in res.results], axis=0)
    attn = np.concatenate([r["attn_sh"] for r in res.results], axis=0)
    return out, attn, res


def bench(inputs, iters=10, n_tok=N, reps=1):
    """Time steady-state sharded executions with inputs staged on-device.

    Returns (best_wall_seconds, all_walls). Upper bound on kernel time:
    includes PJRT dispatch + axon tunnel round-trip.
    """
    import time

    import jax
    from jax.sharding import Mesh, PartitionSpec
    from jax.experimental.shard_map import shard_map
    from concourse import bass2jax, mybir as _mybir

    x = np.ascontiguousarray(inputs["x"], dtype=np.float32)
    ff = np.ascontiguousarray(inputs["fore_feature"], dtype=np.float32)

    nc = _get_nc(n_tok, reps)
    bass2jax.install_neuronx_cc_hook()

    partition_name = nc.partition_id_tensor.name if nc.partition_id_tensor else None
    in_names, out_names, out_avals = [], [], []
    for alloc in nc.m.functions[0].allocations:
        if not isinstance(alloc, _mybir.MemoryLocationSet):
            continue
        name = alloc.memorylocations[0].name
        if alloc.kind == "ExternalInput":
            if name != partition_name:
                in_names.append(name)
        elif alloc.kind == "ExternalOutput":
            out_names.append(name)
            out_avals.append(
                jax.core.ShapedArray(tuple(alloc.tensor_shape), _mybir.dt.np(alloc.dtype))
            )
    n_params = len(in_names)
    all_names = list(in_names) + list(out_names)
    if partition_name is not None:
        all_names.append(partition_name)

    def _body(*args):
        operands = list(args)
        if partition_name is not None:
            operands.append(bass2jax.partition_id_tensor())
        outs = bass2jax._bass_exec_p.bind(
            *operands,
            out_avals=tuple(out_avals),
            in_names=tuple(all_names),
            out_names=tuple(out_names),
            lowering_input_output_aliases=(),
            sim_require_finite=True,
            sim_require_nnan=True,
            nc=nc,
        )
        return tuple(outs)

    devices = jax.devices()[:NCORES]
    mesh = Mesh(np.asarray(devices), ("core",))
    specs = (PartitionSpec("core"),) * (n_params + len(out_names))
    fn = jax.jit(
        shard_map(
            _body, mesh=mesh, in_specs=specs,
            out_specs=(PartitionSpec("core"),) * len(out_names), check_rep=False,
        ),
        keep_unused=True,
        donate_argnums=tuple(range(n_params, n_params + len(out_names))),
    )

    per_core_map = {
        "x_sh": lambda i: x[i * BL : (i + 1) * BL, :n_tok],
        "ff_sh": lambda i: ff[i * BL : (i + 1) * BL, 0],
        "wq": lambda i: inputs["Wq"],
        "bq": lambda i: inputs["bq"],
        "wk": lambda i: inputs["Wk"],
        "wv": lambda i: inputs["Wv"],
        "bv": lambda i: inputs["bv"],
    }
    concat_in = [
        np.concatenate(
            [np.asarray(per_core_map[nm](i), dtype=np.float32) for i in range(NCORES)],
            axis=0,
        )
        for nm in in_names
    ]
    concat_zeros = [
        np.zeros((NCORES * a.shape[0], *a.shape[1:]), a.dtype) for a in out_avals
    ]
    from jax.sharding import NamedSharding

    shard = NamedSharding(mesh, PartitionSpec("core"))
    staged = [jax.device_put(a, shard) for a in concat_in + concat_zeros]

    walls = []
    ins = staged[:n_params]
    outs = fn(*staged)  # compile + warm; donates the zero buffers
    jax.block_until_ready(outs)
    for _ in range(iters):
        t0 = time.perf_counter()
        outs = fn(*ins, *outs)  # recycle output buffers (fully overwritten)
        jax.block_until_ready(outs)
        walls.append(time.perf_counter() - t0)
    return min(walls), walls


def kernel(x, fore_feature, Wq, bq, Wk, bk, Wv, bv, heads):
    inputs = {
        "x": x,
        "fore_feature": fore_feature,
        "Wq": Wq,
        "bq": bq,
        "Wk": Wk,
        "Wv": Wv,
        "bv": bv,
    }
    out, attn, _ = run(inputs)
    return out, attn


# revision 20
# speedup vs baseline: 1.0040x; 1.0040x over previous
"""Trainium2 Bass kernel for nn_Attention (single-query attention variant).

Reference computation (per batch b):
  q = ff @ Wq + bq                      [1, C]   (C=1024, H=16 heads, d=64)
  k = x @ Wk + bk                       [N, C]
  v = x @ Wv + bv                       [N, C]
  s[h, n]    = (1/sqrt(d)) * sum_d q[hd] k[n, hd]
  attn[h, n] = softmax_n(s[h, :])
  out[n, c]  = attn[h(c), n] * v[n, c]

Key algebraic folds used here:
  * k is only consumed through per-head dot products with q, so
    s = x @ Wt + const_h with Wt[c, h] = scale * sum_d Wk[c, hd] q[hd].
    This removes the entire x@Wk matmul (half the FLOPs).
  * const_h (from bk and q) is constant along n, and softmax is
    shift-invariant, so bk drops out of BOTH outputs exactly.
  * softmax needs no max-subtraction: |s| <= ~15 here, exp is safe in fp32.
  * Cross-partition (token) sums of exp(s) ride the PE as ones-vector
    matmuls accumulating into one PSUM bank across all 32 token tiles.

Distribution: pure data-parallel over batch B=16 across 8 cores (2 each).
Matmuls run in float32r (full PE rate, ~1.6e-4 rel err measured on HW).
"""

import sys

sys.path.insert(0, "/opt/trn_rl_repo")

from contextlib import ExitStack

import numpy as np

import concourse.bass as bass
import concourse.tile as tile
from concourse import bacc, mybir
from concourse.bass import ts
from concourse.bass_utils import run_bass_kernel_spmd
from concourse.masks import make_identity

F32 = mybir.dt.float32
F32R = mybir.dt.float32r
F16 = mybir.dt.float16

B, N, C, H, D = 16, 4096, 1024, 16, 64
NCORES = 8
BL = B // NCORES  # batches per core
PT = 128  # tokens per tile
KC = C // 128  # contraction chunks
SEG = 2  # tiles per out-DMA group
SCALE = float(D) ** -0.5


def build_kernel(n_tok: int = N, reps: int = 1, no_transpose: bool = False):
    """Build the per-core SPMD program. n_tok can be reduced for simulation.

    reps > 1 re-emits the steady-state phase; used to measure marginal
    per-iteration hardware time (fixed dispatch overhead cancels).
    """
    nt = n_tok // PT  # token tiles per batch
    ns = nt // SEG

    nc = bacc.Bacc("TRN2", target_bir_lowering=False, debug=False)
    x_d = nc.dram_tensor("x_sh", [BL, n_tok, C], F32, kind="ExternalInput")
    ff_d = nc.dram_tensor("ff_sh", [BL, C], F32, kind="ExternalInput")
    wq_d = nc.dram_tensor("wq", [C, C], F32, kind="ExternalInput")
    bq_d = nc.dram_tensor("bq", [C], F32, kind="ExternalInput")
    wk_d = nc.dram_tensor("wk", [C, C], F32, kind="ExternalInput")
    wv_d = nc.dram_tensor("wv", [C, C], F32, kind="ExternalInput")
    bv_d = nc.dram_tensor("bv", [C], F32, kind="ExternalInput")
    out_d = nc.dram_tensor("out_sh", [BL, n_tok, C], F32, kind="ExternalOutput")
    at_d = nc.dram_tensor("attn_sh", [BL, H, 1, n_tok], F32, kind="ExternalOutput")

    with tile.TileContext(nc) as tc, ExitStack() as ctx:
        _body(ctx, tc, x_d, ff_d, wq_d, bq_d, wk_d, wv_d, bv_d, out_d, at_d, nt, ns, reps, no_transpose)
    nc.compile()
    return nc


def _body(ctx, tc, x_d, ff_d, wq_d, bq_d, wk_d, wv_d, bv_d, out_d, at_d, nt, ns, reps, no_transpose=False):
    n_tok = nt * PT
    nc = tc.nc

    singles = ctx.enter_context(tc.tile_pool(name="singles", bufs=1))
    prol = ctx.enter_context(tc.tile_pool(name="prol", bufs=2))
    xin = ctx.enter_context(tc.tile_pool(name="xin", bufs=2))
    xtp = ctx.enter_context(tc.tile_pool(name="xtp", bufs=2))
    outp = ctx.enter_context(tc.tile_pool(name="outp", bufs=2))
    attp = ctx.enter_context(tc.tile_pool(name="attp", bufs=2))
    stash = ctx.enter_context(tc.tile_pool(name="stash", bufs=nt + 2))

    ps_xt = ctx.enter_context(tc.tile_pool(name="ps_xt", bufs=2, space="PSUM"))
    ps_v = ctx.enter_context(tc.tile_pool(name="ps_v", bufs=3, space="PSUM"))
    ps_s = ctx.enter_context(tc.tile_pool(name="ps_s", bufs=1, space="PSUM"))
    ps_sum = ctx.enter_context(tc.tile_pool(name="ps_sum", bufs=1, space="PSUM"))
    ps_at = ctx.enter_context(tc.tile_pool(name="ps_at", bufs=1, space="PSUM"))

    # ---------------- constants ----------------
    ident = singles.tile([128, 128], F32)
    make_identity(nc, ident)
    ident_h = singles.tile([128, 128], F16)
    nc.scalar.copy(ident_h, ident)
    ones_col = singles.tile([128, 1], F32)
    nc.vector.memset(ones_col, 1.0)

    bv_ap = bv_d.ap()
    bv_rep = singles.tile([128, C], F32)
    nc.gpsimd.dma_start(
        out=bv_rep,
        in_=bass.AP(tensor=bv_ap.tensor, offset=bv_ap.offset, ap=[[0, 128], *bv_ap.ap]),
    )

    # Wv resident in SBUF, fp16 (cast during DMA), [c_in_chunk(part), chunk, c_out]
    wv_sb = singles.tile([128, KC, C], F16)
    nc.gpsimd.dma_start(
        out=wv_sb, in_=wv_d.ap().rearrange("(k p) n -> p k n", p=128)
    )

    # ---------------- prologue: q then Wt (per batch) ----------------
    # ff -> ffT [c(part), chunk, batch]
    ff_sb = singles.tile([BL, C], F32)
    nc.sync.dma_start(out=ff_sb, in_=ff_d.ap())
    ffT_ps = ps_xt.tile([128, KC, BL], F32, tag="xtps")
    for k in range(KC):
        nc.tensor.transpose(ffT_ps[:, k, :], ff_sb[:, ts(k, 128)], ident[:BL, :BL])
    ffT_sb = singles.tile([128, KC, BL], F32R)
    nc.scalar.copy(ffT_sb, ffT_ps)

    # q = ff @ Wq + bq   (accumulated over chunks, f32r full-rate)
    q_ps0 = ps_v.tile([BL, 512], F32, tag="v")
    q_ps1 = ps_v.tile([BL, 512], F32, tag="v")
    q_ps = [q_ps0, q_ps1]
    wq_view = wq_d.ap().rearrange("(k p) n -> k p n", p=128)
    for k in range(KC):
        wq_t = prol.tile([128, C], F32R, tag="wstream")
        nc.sync.dma_start(out=wq_t, in_=wq_view[k].bitcast(F32R))
        for h2 in range(2):
            nc.tensor.matmul(
                q_ps[h2],
                ffT_sb[:, k, :],
                wq_t[:, ts(h2, 512)],
                start=(k == 0),
                stop=(k == KC - 1),
            )
    bq_ap = bq_d.ap()
    bq_sb = singles.tile([BL, C], F32)
    nc.gpsimd.dma_start(
        out=bq_sb,
        in_=bass.AP(tensor=bq_ap.tensor, offset=bq_ap.offset, ap=[[0, BL], *bq_ap.ap]),
    )
    q_sb = singles.tile([BL, C], F32)
    for h2 in range(2):
        nc.vector.tensor_add(q_sb[:, ts(h2, 512)], q_ps[h2], bq_sb[:, ts(h2, 512)])

    # bounce q through DRAM to replicate it across all 128 partitions
    dramp = ctx.enter_context(tc.tile_pool(name="dram", bufs=1, space="DRAM"))
    q_dram = dramp.tile([BL, C], F32)
    nc.sync.dma_start(out=q_dram, in_=q_sb)
    qrepp = ctx.enter_context(tc.tile_pool(name="qrepp", bufs=1))

    # Wt[c, h] = SCALE * sum_d Wk[c, h*D+d] * q[h*D+d], per batch.
    # Computed per batch lazily (batch 0 first so phase A can start early).
    wk_view = wk_d.ap().rearrange("(k p) n -> k p n", p=128)
    wt_sb = singles.tile([128, KC, BL, H], F16)

    def compute_wt(b):
        q_rep = qrepp.tile([128, C], F32, tag="qrep")
        nc.gpsimd.dma_start(
            out=q_rep,
            in_=bass.AP(
                tensor=q_dram.tensor,
                offset=q_dram.offset + b * C,
                ap=[[0, 128], q_dram.ap[1]],
            ),
        )
        for k in range(KC):
            wk_t = prol.tile([128, C], F32, tag="wstream")
            nc.sync.dma_start(out=wk_t, in_=wk_view[k])
            tmp = prol.tile([128, C], F32, tag="wtmp")
            nc.vector.tensor_mul(tmp, wk_t, q_rep)
            red = prol.tile([128, H], F32, tag="wtred")
            nc.vector.reduce_sum(
                out=red,
                in_=tmp.rearrange("p (h d) -> p h d", d=D),
                axis=mybir.AxisListType.X,
            )
            # scale + round-to-f32r in one ACT pass
            nc.scalar.mul(wt_sb[:, k, b, :], red, SCALE)

    # exp(s) stash (f32) and v stash (f16) live across a batch
    exp_sb = singles.tile([128, BL, nt, H], F32)
    attnp = ctx.enter_context(tc.tile_pool(name="attnp", bufs=1))
    rinv = singles.tile([1, BL, H], F32)
    rrep = singles.tile([128, BL, H], F32)

    _xt_frozen = []
    x_view = x_d.ap().rearrange("b (s j p) c -> b s p j c", p=128, j=SEG)
    out_view = out_d.ap().rearrange("b (s j p) c -> b s p j c", p=128, j=SEG)

    for rep in range(reps):
      for b in range(BL):
        if rep == 0:
            compute_wt(b)
        stash_tiles = []
        sums = ps_sum.tile([1, H], F32, tag="sums")

        # ---------------- phase A: stream x, make v + exp(s) ----------------
        for s in range(ns):
            x_t = xin.tile([128, SEG, C], F16)
            nc.gpsimd.dma_start(out=x_t, in_=x_view[b, s])
            for j in range(SEG):
                t = s * SEG + j
                # transpose x tile -> xT [c(part), tok]
                if no_transpose and _xt_frozen:
                    xt_sb = _xt_frozen[0]
                else:
                    xt_sb = xtp.tile([128, KC, 128], F16, tag="xt")
                    _xt_frozen.append(xt_sb)
                for g in range(0 if (no_transpose and len(_xt_frozen) > 1) else 2):
                    xt_ps = ps_xt.tile([128, 4, 128], F16, tag="xtps")
                    for kk in range(4):
                        nc.tensor.transpose(
                            xt_ps[:, kk, :],
                            x_t[:, j, ts(g * 4 + kk, 128)],
                            ident_h,
                        )
                    nc.scalar.copy(xt_sb[:, g * 4 : g * 4 + 4, :], xt_ps)

                # v = x @ Wv (psum, f32r) and s = x @ Wt
                v0 = ps_v.tile([128, 512], F32, tag="v")
                v1 = ps_v.tile([128, 512], F32, tag="v")
                sc = ps_s.tile([128, H], F32, tag="sc")
                for k in range(KC):
                    st = xt_sb[:, k, :]
                    first, last = k == 0, k == KC - 1
                    nc.tensor.matmul(v0, st, wv_sb[:, k, 0:512], start=first, stop=last)
                    nc.tensor.matmul(v1, st, wv_sb[:, k, 512:C], start=first, stop=last)
                    nc.tensor.matmul(
                        sc, st, wt_sb[:, k, b, :], start=first, stop=last
                    )

                # exp(s) -> stash; sum over tokens via ones-matmul into PSUM
                nc.scalar.activation(
                    out=exp_sb[:, b, t, :], in_=sc, func=mybir.ActivationFunctionType.Exp
                )
                nc.tensor.matmul(
                    sums,
                    ones_col,
                    exp_sb[:, b, t, :],
                    start=(t == 0),
                    stop=(t == nt - 1),
                )

                # v + bv -> f16 stash
                st_t = stash.tile([128, C], F16, tag="stash")
                nc.vector.tensor_add(st_t[:, 0:512], v0, bv_rep[:, 0:512])
                nc.vector.tensor_add(st_t[:, 512:C], v1, bv_rep[:, 512:C])
                stash_tiles.append(st_t)

        # ---------------- phase B: normalize, emit attn + out ----------------
        attn_sb = attnp.tile([H, n_tok], F32, tag="attn")
        nc.vector.reciprocal(rinv[:, b, :], sums)
        nc.gpsimd.partition_broadcast(rrep[:, b, :], rinv[:, b, :])

        for s in range(ns):
            o_t = outp.tile([128, SEG, C], F32)
            for j in range(SEG):
                t = s * SEG + j
                at = attp.tile([128, H], F32, tag="at")
                nc.vector.tensor_mul(at, exp_sb[:, b, t, :], rrep[:, b, :])
                atT = ps_at.tile([H, 128], F32, tag="atT")
                nc.tensor.transpose(atT, at, ident)
                nc.scalar.copy(attn_sb[:, ts(t, PT)], atT)
                at_bc = bass.AP(
                    tensor=at.tensor, offset=at.offset, ap=[*at.ap, [0, D]]
                )
                nc.vector.tensor_mul(
                    o_t[:, j, :].rearrange("p (h d) -> p h d", d=D),
                    stash_tiles[t].rearrange("p (h d) -> p h d", d=D),
                    at_bc,
                )
            nc.sync.dma_start(out=out_view[b, s], in_=o_t)
        nc.sync.dma_start(out=at_d.ap()[b, :, 0, :], in_=attn_sb)


_NC_CACHE = {}


def _get_nc(n_tok=N, reps=1, no_transpose=False):
    key = (n_tok, reps, no_transpose)
    if key not in _NC_CACHE:
        _NC_CACHE[key] = build_kernel(n_tok, reps, no_transpose)
    return _NC_CACHE[key]


def run(inputs, n_tok=N, trace=False):
    """Shard inputs, run on 8 cores, gather. Returns (out, attn, results)."""
    x = np.ascontiguousarray(inputs["x"], dtype=np.float32)
    ff = np.ascontiguousarray(inputs["fore_feature"], dtype=np.float32)
    wq = np.ascontiguousarray(inputs["Wq"], dtype=np.float32)
    bq = np.ascontiguousarray(inputs["bq"], dtype=np.float32)
    wk = np.ascontiguousarray(inputs["Wk"], dtype=np.float32)
    wv = np.ascontiguousarray(inputs["Wv"], dtype=np.float32)
    bv = np.ascontiguousarray(inputs["bv"], dtype=np.float32)

    nc = _get_nc(n_tok)
    in_maps = []
    for i in range(NCORES):
        sl = slice(i * BL, (i + 1) * BL)
        in_maps.append(
            {
                "x_sh": np.ascontiguousarray(x[sl, :n_tok]),
                "ff_sh": np.ascontiguousarray(ff[sl, 0]),
                "wq": wq,
                "bq": bq,
                "wk": wk,
                "wv": wv,
                "bv": bv,
            }
        )
    res = run_bass_kernel_spmd(
        nc, in_maps, core_ids=list(range(NCORES)), trace=trace
    )
    out = np.concatenate([r["out_sh"] for r in res.results], axis=0)
    attn = np.concatenate([r["attn_sh"] for r in res.results], axis=0)
    return out, attn, res


def bench(inputs, iters=10, n_tok=N, reps=1, no_transpose=False):
    """Time steady-state sharded executions with inputs staged on-device.

    Returns (best_wall_seconds, all_walls). Upper bound on kernel time:
    includes PJRT dispatch + axon tunnel round-trip.
    """
    import time

    import jax
    from jax.sharding import Mesh, PartitionSpec
    from jax.experimental.shard_map import shard_map
    from concourse import bass2jax, mybir as _mybir

    x = np.ascontiguousarray(inputs["x"], dtype=np.float32)
    ff = np.ascontiguousarray(inputs["fore_feature"], dtype=np.float32)

    nc = _get_nc(n_tok, reps, no_transpose)
    bass2jax.install_neuronx_cc_hook()

    partition_name = nc.partition_id_tensor.name if nc.partition_id_tensor else None
    in_names, out_names, out_avals = [], [], []
    for alloc in nc.m.functions[0].allocations:
        if not isinstance(alloc, _mybir.MemoryLocationSet):
            continue
        name = alloc.memorylocations[0].name
        if alloc.kind == "ExternalInput":
            if name != partition_name:
                in_names.append(name)
        elif alloc.kind == "ExternalOutput":
            out_names.append(name)
            out_avals.append(
                jax.core.ShapedArray(tuple(alloc.tensor_shape), _mybir.dt.np(alloc.dtype))
            )
    n_params = len(in_names)
    all_names = list(in_names) + list(out_names)
    if partition_name is not None:
        all_names.append(partition_name)

    def _body(*args):
        operands = list(args)
        if partition_name is not None:
            operands.append(bass2jax.partition_id_tensor())
        outs = bass2jax._bass_exec_p.bind(
            *operands,
            out_avals=tuple(out_avals),
            in_names=tuple(all_names),
            out_names=tuple(out_names),
            lowering_input_output_aliases=(),
            sim_require_finite=True,
            sim_require_nnan=True,
            nc=nc,
        )
        return tuple(outs)

    devices = jax.devices()[:NCORES]
    mesh = Mesh(np.asarray(devices), ("core",))
    specs = (PartitionSpec("core"),) * (n_params + len(out_names))
    fn = jax.jit(
        shard_map(
            _body, mesh=mesh, in_specs=specs,
            out_specs=(PartitionSpec("core"),) * len(out_names), check_rep=False,
        ),
        keep_unused=True,
        donate_argnums=tuple(range(n_params, n_params + len(out_names))),
    )

    per_core_map = {
        "x_sh": lambda i: x[i * BL : (i + 1) * BL, :n_tok],
        "ff_sh": lambda i: ff[i * BL : (i + 1) * BL, 0],
        "wq": lambda i: inputs["Wq"],
        "bq": lambda i: inputs["bq"],
        "wk": lambda i: inputs["Wk"],
        "wv": lambda i: inputs["Wv"],
        "bv": lambda i: inputs["bv"],
    }
    concat_in = [
        np.concatenate(
            [np.asarray(per_core_map[nm](i), dtype=np.float32) for i in range(NCORES)],
            axis=0,
        )
        for nm in in_names
    ]
    concat_zeros = [
        np.zeros((NCORES * a.shape[0], *a.shape[1:]), a.dtype) for a in out_avals
    ]
    from jax.sharding import NamedSharding

    shard = NamedSharding(mesh, PartitionSpec("core"))
    staged = [jax.device_put(a, shard) for a in concat_in + concat_zeros]

    walls = []
    ins = staged[:n_params]
    outs = fn(*staged)  # compile + warm; donates the zero buffers
    jax.block_until_ready(outs)
    for _ in range(iters):
        t0 = time.perf_counter()
        outs = fn(*ins, *outs)  # recycle output buffers (fully overwritten)
        jax.block_until_ready(outs)
        walls.append(time.perf_counter() - t0)
    return min(walls), walls


def kernel(x, fore_feature, Wq, bq, Wk, bk, Wv, bv, heads):
    inputs = {
        "x": x,
        "fore_feature": fore_feature,
        "Wq": Wq,
        "bq": bq,
        "Wk": Wk,
        "Wv": Wv,
        "bv": bv,
    }
    out, attn, _ = run(inputs)
    return out, attn


# revision 23
# speedup vs baseline: 220.2295x; 219.3510x over previous
"""Trainium2 Bass kernel for nn_Attention (single-query attention variant).

Reference computation (per batch b):
  q = ff @ Wq + bq                      [1, C]   (C=1024, H=16 heads, d=64)
  k = x @ Wk + bk                       [N, C]
  v = x @ Wv + bv                       [N, C]
  s[h, n]    = (1/sqrt(d)) * sum_d q[hd] k[n, hd]
  attn[h, n] = softmax_n(s[h, :])
  out[n, c]  = attn[h(c), n] * v[n, c]

Key algebraic folds used here:
  * k is only consumed through per-head dot products with q, so
    s = x @ Wt + const_h with Wt[c, h] = scale * sum_d Wk[c, hd] q[hd].
    This removes the entire x@Wk matmul (half the FLOPs).
  * const_h (from bk and q) is constant along n, and softmax is
    shift-invariant, so bk drops out of BOTH outputs exactly.
  * softmax needs no max-subtraction: |s| <= ~15 here, exp is safe in fp32.
  * Cross-partition (token) sums of exp(s) ride the PE as ones-vector
    matmuls accumulating into one PSUM bank across all 32 token tiles.

Distribution: pure data-parallel over batch B=16 across 8 cores (2 each).
Matmuls run in float32r (full PE rate, ~1.6e-4 rel err measured on HW).
"""

import sys

sys.path.insert(0, "/opt/trn_rl_repo")

from contextlib import ExitStack

import numpy as np

import concourse.bass as bass
import concourse.tile as tile
from concourse import bacc, mybir
from concourse.bass import ts
from concourse.bass_utils import run_bass_kernel_spmd
from concourse.masks import make_identity

F32 = mybir.dt.float32
F32R = mybir.dt.float32r
F16 = mybir.dt.float16

B, N, C, H, D = 16, 4096, 1024, 16, 64
NCORES = 8
BL = B // NCORES  # batches per core
PT = 128  # tokens per tile
KC = C // 128  # contraction chunks
SEG = 2  # tiles per out-DMA group
SCALE = float(D) ** -0.5


def build_kernel(n_tok: int = N, reps: int = 1, no_transpose: bool = False):
    """Build the per-core SPMD program. n_tok can be reduced for simulation.

    reps > 1 re-emits the steady-state phase; used to measure marginal
    per-iteration hardware time (fixed dispatch overhead cancels).
    """
    nt = n_tok // PT  # token tiles per batch
    ns = nt // SEG

    nc = bacc.Bacc("TRN2", target_bir_lowering=False, debug=False)
    x_d = nc.dram_tensor("x_sh", [BL, n_tok, C], F32, kind="ExternalInput")
    ff_d = nc.dram_tensor("ff_sh", [BL, C], F32, kind="ExternalInput")
    wq_d = nc.dram_tensor("wq", [C, C], F32, kind="ExternalInput")
    bq_d = nc.dram_tensor("bq", [C], F32, kind="ExternalInput")
    wk_d = nc.dram_tensor("wk", [C, C], F32, kind="ExternalInput")
    wv_d = nc.dram_tensor("wv", [C, C], F32, kind="ExternalInput")
    bv_d = nc.dram_tensor("bv", [C], F32, kind="ExternalInput")
    out_d = nc.dram_tensor("out_sh", [BL, n_tok, C], F32, kind="ExternalOutput")
    at_d = nc.dram_tensor("attn_sh", [BL, H, 1, n_tok], F32, kind="ExternalOutput")

    with tile.TileContext(nc) as tc, ExitStack() as ctx:
        _body(ctx, tc, x_d, ff_d, wq_d, bq_d, wk_d, wv_d, bv_d, out_d, at_d, nt, ns, reps, no_transpose)
    nc.compile()
    return nc


def _body(ctx, tc, x_d, ff_d, wq_d, bq_d, wk_d, wv_d, bv_d, out_d, at_d, nt, ns, reps, no_transpose=False):
    n_tok = nt * PT
    nc = tc.nc

    singles = ctx.enter_context(tc.tile_pool(name="singles", bufs=1))
    prol = ctx.enter_context(tc.tile_pool(name="prol", bufs=2))
    xin = ctx.enter_context(tc.tile_pool(name="xin", bufs=3))
    xtp = ctx.enter_context(tc.tile_pool(name="xtp", bufs=3))
    outp = ctx.enter_context(tc.tile_pool(name="outp", bufs=2))
    attp = ctx.enter_context(tc.tile_pool(name="attp", bufs=2))
    stash = ctx.enter_context(tc.tile_pool(name="stash", bufs=nt + 2))

    ps_xt = ctx.enter_context(tc.tile_pool(name="ps_xt", bufs=2, space="PSUM"))
    ps_v = ctx.enter_context(tc.tile_pool(name="ps_v", bufs=3, space="PSUM"))
    ps_s = ctx.enter_context(tc.tile_pool(name="ps_s", bufs=1, space="PSUM"))
    ps_sum = ctx.enter_context(tc.tile_pool(name="ps_sum", bufs=1, space="PSUM"))
    ps_at = ctx.enter_context(tc.tile_pool(name="ps_at", bufs=1, space="PSUM"))

    # ---------------- constants ----------------
    ident = singles.tile([128, 128], F32)
    make_identity(nc, ident)
    ident_h = singles.tile([128, 128], F16)
    nc.scalar.copy(ident_h, ident)
    ones_col = singles.tile([128, 1], F32)
    nc.vector.memset(ones_col, 1.0)

    bv_ap = bv_d.ap()
    bv_rep = singles.tile([128, C], F32)
    nc.gpsimd.dma_start(
        out=bv_rep,
        in_=bass.AP(tensor=bv_ap.tensor, offset=bv_ap.offset, ap=[[0, 128], *bv_ap.ap]),
    )

    # Wv resident in SBUF, fp16 (cast during DMA), [c_in_chunk(part), chunk, c_out]
    wv_sb = singles.tile([128, KC, C], F16)
    nc.gpsimd.dma_start(
        out=wv_sb, in_=wv_d.ap().rearrange("(k p) n -> p k n", p=128)
    )

    # ---------------- prologue: q then Wt (per batch) ----------------
    # ff -> ffT [c(part), chunk, batch]
    ff_sb = singles.tile([BL, C], F32)
    nc.sync.dma_start(out=ff_sb, in_=ff_d.ap())
    ffT_ps = ps_xt.tile([128, KC, BL], F32, tag="xtps")
    for k in range(KC):
        nc.tensor.transpose(ffT_ps[:, k, :], ff_sb[:, ts(k, 128)], ident[:BL, :BL])
    ffT_sb = singles.tile([128, KC, BL], F32R)
    nc.scalar.copy(ffT_sb, ffT_ps)

    # q = ff @ Wq + bq   (accumulated over chunks, f32r full-rate)
    q_ps0 = ps_v.tile([BL, 512], F32, tag="v")
    q_ps1 = ps_v.tile([BL, 512], F32, tag="v")
    q_ps = [q_ps0, q_ps1]
    wq_view = wq_d.ap().rearrange("(k p) n -> k p n", p=128)
    for k in range(KC):
        wq_t = prol.tile([128, C], F32R, tag="wstream")
        nc.sync.dma_start(out=wq_t, in_=wq_view[k].bitcast(F32R))
        for h2 in range(2):
            nc.tensor.matmul(
                q_ps[h2],
                ffT_sb[:, k, :],
                wq_t[:, ts(h2, 512)],
                start=(k == 0),
                stop=(k == KC - 1),
            )
    bq_ap = bq_d.ap()
    bq_sb = singles.tile([BL, C], F32)
    nc.gpsimd.dma_start(
        out=bq_sb,
        in_=bass.AP(tensor=bq_ap.tensor, offset=bq_ap.offset, ap=[[0, BL], *bq_ap.ap]),
    )
    q_sb = singles.tile([BL, C], F32)
    for h2 in range(2):
        nc.vector.tensor_add(q_sb[:, ts(h2, 512)], q_ps[h2], bq_sb[:, ts(h2, 512)])

    # bounce q through DRAM to replicate it across all 128 partitions
    dramp = ctx.enter_context(tc.tile_pool(name="dram", bufs=1, space="DRAM"))
    q_dram = dramp.tile([BL, C], F32)
    nc.sync.dma_start(out=q_dram, in_=q_sb)
    qrepp = ctx.enter_context(tc.tile_pool(name="qrepp", bufs=1))

    # Wt[c, h] = SCALE * sum_d Wk[c, h*D+d] * q[h*D+d], per batch.
    # Computed per batch lazily (batch 0 first so phase A can start early).
    wk_view = wk_d.ap().rearrange("(k p) n -> k p n", p=128)
    wt_sb = singles.tile([128, KC, BL, H], F16)

    def compute_wt(b):
        q_rep = qrepp.tile([128, C], F32, tag="qrep")
        nc.gpsimd.dma_start(
            out=q_rep,
            in_=bass.AP(
                tensor=q_dram.tensor,
                offset=q_dram.offset + b * C,
                ap=[[0, 128], q_dram.ap[1]],
            ),
        )
        for k in range(KC):
            wk_t = prol.tile([128, C], F32, tag="wstream")
            nc.sync.dma_start(out=wk_t, in_=wk_view[k])
            tmp = prol.tile([128, C], F32, tag="wtmp")
            nc.vector.tensor_mul(tmp, wk_t, q_rep)
            red = prol.tile([128, H], F32, tag="wtred")
            nc.vector.reduce_sum(
                out=red,
                in_=tmp.rearrange("p (h d) -> p h d", d=D),
                axis=mybir.AxisListType.X,
            )
            # scale + round-to-f32r in one ACT pass
            nc.scalar.mul(wt_sb[:, k, b, :], red, SCALE)

    # exp(s) stash (f32) and v stash (f16) live across a batch
    exp_sb = singles.tile([128, BL, nt, H], F32)
    attnp = ctx.enter_context(tc.tile_pool(name="attnp", bufs=1))
    rinv = singles.tile([1, BL, H], F32)
    rrep = singles.tile([128, BL, H], F32)

    _xt_frozen = []
    x_view = x_d.ap().rearrange("b (s j p) c -> b s p j c", p=128, j=SEG)
    out_view = out_d.ap().rearrange("b (s j p) c -> b s p j c", p=128, j=SEG)

    for rep in range(reps):
      for b in range(BL):
        if rep == 0:
            compute_wt(b)
        stash_tiles = []
        sums = ps_sum.tile([1, H], F32, tag="sums")

        # ---------------- phase A: stream x, make v + exp(s) ----------------
        for s in range(ns):
            x_t = xin.tile([128, SEG, C], F16)
            nc.gpsimd.dma_start(out=x_t, in_=x_view[b, s])
            for j in range(SEG):
                t = s * SEG + j
                # transpose x tile -> xT [c(part), tok]
                if no_transpose and _xt_frozen:
                    xt_sb = _xt_frozen[0]
                else:
                    xt_sb = xtp.tile([128, KC, 128], F16, tag="xt")
                    _xt_frozen.append(xt_sb)
                if not (no_transpose and len(_xt_frozen) > 1):
                    xt_ps = ps_xt.tile([128, KC, 128], F16, tag="xtps")
                    for kk in range(KC):
                        nc.tensor.transpose(
                            xt_ps[:, kk, :],
                            x_t[:, j, ts(kk, 128)],
                            ident_h,
                        )
                    nc.scalar.copy(xt_sb, xt_ps)

                # v = x @ Wv (psum, f32r) and s = x @ Wt
                v0 = ps_v.tile([128, 512], F32, tag="v")
                v1 = ps_v.tile([128, 512], F32, tag="v")
                sc = ps_s.tile([128, H], F32, tag="sc")
                for k in range(KC):
                    st = xt_sb[:, k, :]
                    first, last = k == 0, k == KC - 1
                    nc.tensor.matmul(v0, st, wv_sb[:, k, 0:512], start=first, stop=last)
                    nc.tensor.matmul(v1, st, wv_sb[:, k, 512:C], start=first, stop=last)
                    nc.tensor.matmul(
                        sc, st, wt_sb[:, k, b, :], start=first, stop=last
                    )

                # exp(s) -> stash; sum over tokens via ones-matmul into PSUM
                nc.scalar.activation(
                    out=exp_sb[:, b, t, :], in_=sc, func=mybir.ActivationFunctionType.Exp
                )
                nc.tensor.matmul(
                    sums,
                    ones_col,
                    exp_sb[:, b, t, :],
                    start=(t == 0),
                    stop=(t == nt - 1),
                )

                # v + bv -> f16 stash
                st_t = stash.tile([128, C], F16, tag="stash")
                nc.vector.tensor_add(st_t[:, 0:512], v0, bv_rep[:, 0:512])
                nc.vector.tensor_add(st_t[:, 512:C], v1, bv_rep[:, 512:C])
                stash_tiles.append(st_t)

        # ---------------- phase B: normalize, emit attn + out ----------------
        attn_sb = attnp.tile([H, n_tok], F32, tag="attn")
        nc.vector.reciprocal(rinv[:, b, :], sums)
        nc.gpsimd.partition_broadcast(rrep[:, b, :], rinv[:, b, :])

        for s in range(ns):
            o_t = outp.tile([128, SEG, C], F32)
            for j in range(SEG):
                t = s * SEG + j
                at = attp.tile([128, H], F32, tag="at")
                nc.vector.tensor_mul(at, exp_sb[:, b, t, :], rrep[:, b, :])
                atT = ps_at.tile([H, 128], F32, tag="atT")
                nc.tensor.transpose(atT, at, ident)
                nc.scalar.copy(attn_sb[:, ts(t, PT)], atT)
                at_bc = bass.AP(
                    tensor=at.tensor, offset=at.offset, ap=[*at.ap, [0, D]]
                )
                nc.vector.tensor_mul(
                    o_t[:, j, :].rearrange("p (h d) -> p h d", d=D),
                    stash_tiles[t].rearrange("p (h d) -> p h d", d=D),
                    at_bc,
                )
            nc.sync.dma_start(out=out_view[b, s], in_=o_t)
        nc.sync.dma_start(out=at_d.ap()[b, :, 0, :], in_=attn_sb)


_NC_CACHE = {}


def _get_nc(n_tok=N, reps=1, no_transpose=False):
    key = (n_tok, reps, no_transpose)
    if key not in _NC_CACHE:
        _NC_CACHE[key] = build_kernel(n_tok, reps, no_transpose)
    return _NC_CACHE[key]


def run(inputs, n_tok=N, trace=False):
    """Shard inputs, run on 8 cores, gather. Returns (out, attn, results)."""
    x = np.ascontiguousarray(inputs["x"], dtype=np.float32)
    ff = np.ascontiguousarray(inputs["fore_feature"], dtype=np.float32)
    wq = np.ascontiguousarray(inputs["Wq"], dtype=np.float32)
    bq = np.ascontiguousarray(inputs["bq"], dtype=np.float32)
    wk = np.ascontiguousarray(inputs["Wk"], dtype=np.float32)
    wv = np.ascontiguousarray(inputs["Wv"], dtype=np.float32)
    bv = np.ascontiguousarray(inputs["bv"], dtype=np.float32)

    nc = _get_nc(n_tok)
    in_maps = []
    for i in range(NCORES):
        sl = slice(i * BL, (i + 1) * BL)
        in_maps.append(
            {
                "x_sh": np.ascontiguousarray(x[sl, :n_tok]),
                "ff_sh": np.ascontiguousarray(ff[sl, 0]),
                "wq": wq,
                "bq": bq,
                "wk": wk,
                "wv": wv,
                "bv": bv,
            }
        )
    res = run_bass_kernel_spmd(
        nc, in_maps, core_ids=list(range(NCORES)), trace=trace
    )
    out = np.concatenate([r["out_sh"] for r in res.results], axis=0)
    attn = np.concatenate([r["attn_sh"] for r in res.results], axis=0)
    return out, attn, res


def bench(inputs, iters=10, n_tok=N, reps=1, no_transpose=False):
    """Time steady-state sharded executions with inputs staged on-device.

    Returns (best_wall_seconds, all_walls). Upper bound on kernel time:
    includes PJRT dispatch + axon tunnel round-trip.
    """
    import time

    import jax
    from jax.sharding import Mesh, PartitionSpec
    from jax.experimental.shard_map import shard_map
    from concourse import bass2jax, mybir as _mybir

    x = np.ascontiguousarray(inputs["x"], dtype=np.float32)
    ff = np.ascontiguousarray(inputs["fore_feature"], dtype=np.float32)

    nc = _get_nc(n_tok, reps, no_transpose)
    bass2jax.install_neuronx_cc_hook()

    partition_name = nc.partition_id_tensor.name if nc.partition_id_tensor else None
    in_names, out_names, out_avals = [], [], []
    for alloc in nc.m.functions[0].allocations:
        if not isinstance(alloc, _mybir.MemoryLocationSet):
            continue
        name = alloc.memorylocations[0].name
        if alloc.kind == "ExternalInput":
            if name != partition_name:
                in_names.append(name)
        elif alloc.kind == "ExternalOutput":
            out_names.append(name)
            out_avals.append(
                jax.core.ShapedArray(tuple(alloc.tensor_shape), _mybir.dt.np(alloc.dtype))
            )
    n_params = len(in_names)
    all_names = list(in_names) + list(out_names)
    if partition_name is not None:
        all_names.append(partition_name)

    def _body(*args):
        operands = list(args)
        if partition_name is not None:
            operands.append(bass2jax.partition_id_tensor())
        outs = bass2jax._bass_exec_p.bind(
            *operands,
            out_avals=tuple(out_avals),
            in_names=tuple(all_names),
            out_names=tuple(out_names),
            lowering_input_output_aliases=(),
            sim_require_finite=True,
            sim_require_nnan=True,
            nc=nc,
        )
        return tuple(outs)

    devices = jax.devices()[:NCORES]
    mesh = Mesh(np.asarray(devices), ("core",))
    specs = (PartitionSpec("core"),) * (n_params + len(out_names))
    fn = jax.jit(
        shard_map(
            _body, mesh=mesh, in_specs=specs,
            out_specs=(PartitionSpec("core"),) * len(out_names), check_rep=False,
        ),
        keep_unused=True,
        donate_argnums=tuple(range(n_params, n_params + len(out_names))),
    )

    per_core_map = {
        "x_sh": lambda i: x[i * BL : (i + 1) * BL, :n_tok],
        "ff_sh": lambda i: ff[i * BL : (i + 1) * BL, 0],
        "wq": lambda i: inputs["Wq"],
        "bq": lambda i: inputs["bq"],
        "wk": lambda i: inputs["Wk"],
        "wv": lambda i: inputs["Wv"],
        "bv": lambda i: inputs["bv"],
    }
    concat_in = [
        np.concatenate(
            [np.asarray(per_core_map[nm](i), dtype=np.float32) for i in range(NCORES)],
            axis=0,
        )
        for nm in in_names
    ]
    concat_zeros = [
        np.zeros((NCORES * a.shape[0], *a.shape[1:]), a.dtype) for a in out_avals
    ]
    from jax.sharding import NamedSharding

    shard = NamedSharding(mesh, PartitionSpec("core"))
    staged = [jax.device_put(a, shard) for a in concat_in + concat_zeros]

    walls = []
    ins = staged[:n_params]
    outs = fn(*staged)  # compile + warm; donates the zero buffers
    jax.block_until_ready(outs)

    state = {"outs": outs}

    def call():
        t0 = time.perf_counter()
        state["outs"] = fn(*ins, *state["outs"])
        jax.block_until_ready(state["outs"])
        return time.perf_counter() - t0

    if iters == 0:
        return call
    for _ in range(iters):
        walls.append(call())
    return min(walls), walls


def kernel(x, fore_feature, Wq, bq, Wk, bk, Wv, bv, heads):
    inputs = {
        "x": x,
        "fore_feature": fore_feature,
        "Wq": Wq,
        "bq": bq,
        "Wk": Wk,
        "Wv": Wv,
        "bv": bv,
    }
    out, attn, _ = run(inputs)
    return out, attn
